# revision 28
# baseline (speedup 1.0000x reference)
"""Batched CRF Viterbi decode on 8 TRN2 NeuronCores.

Data-parallel over batch (16 sequences per core). The 511-step sequential
max-plus recurrence is split into TWO independent 255/256-step chains that
run concurrently on each core: a forward chain (part_t for t=1..255) and a
backward chain (beta_t for t=510..255, beta = best tail score including the
final ->END hop). All sequence lengths are >= 256, so every sequence end
falls in the backward half; ends are handled by an inject column (51st
column of the backward score tile, = trans[i,END] at t==last_pos) and a
per-(b,t) clamp (scalar_tensor_tensor min) that pins the state to NEG on
steps past the sequence end. The host stitches the halves at t=255 via
argmax(part+beta) and reconstructs the reference-equivalent decode.

Both chains recenter per (b,t) by c=max_j feats[b,t,j] (host-folded), so
states stay small enough to travel through the PE as float16 (1 cycle/row).
Per chain per step: Pool builds ft16 in SBUF; an fp16 identity matmul moves
it to PSUM (issued first in PE order so reduce sem-waits stay precise); one
fp16 K=128 matmul (REP128 @ zero-embedded state) accumulates the gathered
state; DVE reduces max over the score axis into the fp32 history and
re-embeds the fp16 state (backward: with the clamp via stt).
"""

import numpy as np

B, S, T = 128, 512, 50
NCORES = 8
BL = B // NCORES          # 16 sequences per core
JG, JL = 8, 7             # 8 groups x 7 tags = 56 padded tags
TP = JG * JL              # 56
NF = JL * TP              # fwd transP columns
MID = S // 2              # 256
NF_F = MID * JL           # fwd feats cols (t=0..255)
NSF = MID - 1             # 255 fwd steps (t=1..255)
NSB = MID                 # 256 bwd steps (t=510..255 plus seed t=511)
CB = T + 1                # bwd score cols: 50 + inject
START, END = T - 2, T - 1
NEG = np.float32(-25000.0)  # padding; must survive float16
BIG = np.float32(60000.0)


def _host_prep2(feats, mask, transitions):
    """Build per-core device input arrays for both chains."""
    f = np.ascontiguousarray(feats, dtype=np.float32)         # (B,S,T)
    tr = np.ascontiguousarray(transitions, dtype=np.float32)  # (T,T)
    c = f.max(axis=2)                                         # (B,S)
    lengths = mask.astype(np.int64).sum(axis=1)
    lp = lengths - 1                                          # in [255,511]

    trp = np.full((TP, TP), NEG, dtype=np.float32)
    trp[:T, :T] = tr

    k = np.arange(128)
    # fwd: transP[p=(jg,b), (jl, i)] = trp[i, jg*7+jl]
    transP = np.empty((128, JL, TP), dtype=np.float32)
    for g in range(JG):
        transP[g * BL:(g + 1) * BL] = trp[:, g * JL:(g + 1) * JL].T[None]
    transP = np.ascontiguousarray(transP.reshape(128, NF))
    # bwd: transPB[p=(ig,b), (il, j)] = trp[ig*7+il, j]  (j = 0..49)
    transPB = np.empty((128, JL, T), dtype=np.float32)
    for g in range(JG):
        transPB[g * BL:(g + 1) * BL] = trp[g * JL:(g + 1) * JL, :T][None]
    transPB = np.ascontiguousarray(transPB.reshape(128, JL * T))

    REP128 = (k[:, None] % BL == k[None, :] % BL).astype(np.float16)
    G = ((np.arange(TP)[None, :] // JL) == (k[:, None] // BL)).astype(np.float32)

    # bwd recentering: cb[b,tau] = c for real steps, 0 for masked
    cb = np.where(np.arange(S)[None, :] <= lp[:, None], c, 0.0).astype(np.float32)

    per_core = []
    for ci in range(NCORES):
        sl = slice(ci * BL, (ci + 1) * BL)
        fb, cbf, cbb, lpb = f[sl], c[sl], cb[sl], lp[sl]      # per-core views

        # ---- forward arrays (t = 0..MID-1) ----
        fp = np.zeros((BL, MID, TP), dtype=np.float32)
        fp[:, :, :T] = fb[:, :MID, :] - cbf[:, :MID, None]
        fa = fp.reshape(BL, MID, JG, JL).transpose(2, 0, 1, 3).reshape(128, NF_F)
        part0 = np.full((BL, TP), NEG, dtype=np.float32)
        part0[:, :T] = fb[:, 0, :] + tr[START][None, :] - cbf[:, 0:1]
        mw0 = np.repeat(part0[None], JG, axis=0).reshape(128, TP) * G

        # ---- backward arrays: step s=1..NSB computes t = S-1-s ----
        svec = np.arange(1, NSB + 1)
        tvec = S - 1 - svec                                   # 510..255
        # featsB[p=(ig,b), (s-1)*T + j] = feats[b, t(s)+1, j] - cb[b, t(s)+1]
        fB = (fb[:, tvec + 1, :] - cbb[np.arange(BL)[:, None], tvec + 1][:, :, None])
        featsB = np.broadcast_to(fB[None], (JG, BL, NSB, T)) \
            .reshape(128, NSB * T).astype(np.float16)
        # injA[p=(ig,b), (s-1)*7+il] = trans[i(ig,il),END] if t(s)==lp else NEG
        trE = np.full(TP, NEG, dtype=np.float32)
        trE[:T] = tr[:, END]
        hit = (tvec[None, :] == lpb[:, None])                 # (BL, NSB)
        injA = np.where(
            hit[None, :, :, None],                            # (1,BL,NSB,1)
            trE.reshape(JG, 1, 1, JL),                        # (JG,1,1,JL)
            NEG).transpose(0, 1, 2, 3).reshape(JG * BL, NSB * JL)
        injA = np.ascontiguousarray(injA.astype(np.float16))
        # clampB[p, s-1] = NEG if t(s) > lp else BIG
        clampB = np.where((tvec[None, :] > lpb[:, None])[None],
                          NEG, BIG)
        clampB = np.broadcast_to(clampB, (JG, BL, NSB)) \
            .reshape(128, NSB).astype(np.float32)
        # seed beta'_{511}
        seed = np.where((lpb == S - 1)[:, None], trE[None, :T], NEG)
        seedp = np.full((BL, TP), NEG, dtype=np.float32)
        seedp[:, :T] = seed
        mw0B = np.repeat(seedp[None], JG, axis=0).reshape(128, TP) * G

        per_core.append({
            "feats_arr": np.ascontiguousarray(fa.astype(np.float16)),
            "transP": transP,
            "transPB": transPB,
            "featsB": featsB,
            "injA": injA,
            "clampB": np.ascontiguousarray(clampB),
            "I128": np.eye(128, dtype=np.float16),
            "REP128": REP128,
            "G": G,
            "mw0": np.ascontiguousarray(mw0.astype(np.float16)),
            "mw0B": np.ascontiguousarray(mw0B.astype(np.float16)),
        })
    return per_core


def build_bass():
    import concourse.bacc as bacc
    import concourse.mybir as mybir
    import concourse.tile as tile

    f32 = mybir.dt.float32
    f16 = mybir.dt.float16
    nc = bacc.Bacc("TRN2", target_bir_lowering=False, debug=False,
                   num_devices=NCORES)

    feats_d = nc.declare_dram_parameter("feats_arr", [128, NF_F], f16, isOutput=False)
    transP_d = nc.declare_dram_parameter("transP", [128, NF], f32, isOutput=False)
    transPB_d = nc.declare_dram_parameter("transPB", [128, JL * T], f32, isOutput=False)
    featsB_d = nc.declare_dram_parameter("featsB", [128, NSB * T], f16, isOutput=False)
    injA_d = nc.declare_dram_parameter("injA", [128, NSB * JL], f16, isOutput=False)
    clampB_d = nc.declare_dram_parameter("clampB", [128, NSB], f32, isOutput=False)
    i128_d = nc.declare_dram_parameter("I128", [128, 128], f16, isOutput=False)
    rep_d = nc.declare_dram_parameter("REP128", [128, 128], f16, isOutput=False)
    g_d = nc.declare_dram_parameter("G", [128, TP], f32, isOutput=False)
    mw0_d = nc.declare_dram_parameter("mw0", [128, TP], f16, isOutput=False)
    mw0B_d = nc.declare_dram_parameter("mw0B", [128, TP], f16, isOutput=False)
    hist_d = nc.declare_dram_parameter("hist", [128, NSF * JL], f16, isOutput=True)
    histB_d = nc.declare_dram_parameter("histB", [128, NSB * JL], f16, isOutput=True)

    with tile.TileContext(nc) as tc:
        with (
            tc.tile_pool(name="static", bufs=1) as sp,
            tc.tile_pool(name="state", bufs=6) as st,
            tc.tile_pool(name="psum", bufs=2, space="PSUM") as pp,
        ):
            def load(handle, shape, dt, tag):
                t_ = sp.tile(shape, dt, tag=tag)
                nc.sync.dma_start(out=t_[:, :], in_=handle[:, :])
                return t_
            # small static loads first so they land before the bulk arrays
            transP_sb = load(transP_d, [128, NF], f32, "trP")
            transPB_sb = load(transPB_d, [128, JL * T], f32, "trPB")
            clampB_sb = load(clampB_d, [128, NSB], f32, "clampB")
            i128_sb = load(i128_d, [128, 128], f16, "i128")
            rep_sb = load(rep_d, [128, 128], f16, "rep")
            # bulk per-step arrays as 8 independent tiles (32 steps each) so
            # the first builds only wait on chunk 0, not the whole array
            FCH, BCH, ICH = NF_F // 8, NSB * T // 8, NSB * JL // 8
            feats_sbs, featsB_sbs, injA_sbs = [], [], []
            for ci8 in range(8):
                tb = sp.tile([128, BCH], f16, tag="fB%d" % ci8)
                nc.sync.dma_start(out=tb[:, :],
                                  in_=featsB_d[:, ci8 * BCH:(ci8 + 1) * BCH])
                featsB_sbs.append(tb)
                tf = sp.tile([128, FCH], f16, tag="fF%d" % ci8)
                nc.sync.dma_start(out=tf[:, :],
                                  in_=feats_d[:, ci8 * FCH:(ci8 + 1) * FCH])
                feats_sbs.append(tf)
                ti = sp.tile([128, ICH], f16, tag="fI%d" % ci8)
                nc.sync.dma_start(out=ti[:, :],
                                  in_=injA_d[:, ci8 * ICH:(ci8 + 1) * ICH])
                injA_sbs.append(ti)
            g_sb = sp.tile([128, JG, JL], f32)
            nc.sync.dma_start(out=g_sb[:, :, :], in_=g_d[:, :].rearrange(
                "p (a b) -> p a b", a=JG))

            hist_sb = sp.tile([128, NSF * JL], f16)
            histB_sb = sp.tile([128, NSB * JL], f16)

            mfwF = st.tile([128, TP], f16, tag="mfwF")
            nc.sync.dma_start(out=mfwF[:, :], in_=mw0_d[:, :])
            mfwB = st.tile([128, TP], f16, tag="mfwB")
            nc.sync.dma_start(out=mfwB[:, :], in_=mw0B_d[:, :])

            transP_v = transP_sb[:, :].rearrange("p (a b) -> p a b", a=JL)
            transPB_v = transPB_sb[:, :].rearrange("p (a b) -> p a b", a=JL)

            from concourse.tile_rust import add_dep_helper

            ftF, ftB, cF, cB = {}, {}, {}, {}
            gate = [None]

            def build_F(ss):
                # fwd ft build split across the idle Act engine (4 rows via
                # Copy-activation with per-partition feats bias) and Pool
                # (3 rows), relieving the Pool bottleneck
                t_ = st.tile([128, JL, T], f16, tag="ftF%d" % (ss % 2))
                fsb, fo = feats_sbs[ss // 32], (ss % 32) * JL
                for jl in range(5):
                    nc.scalar.activation(
                        out=t_[:, jl:jl + 1, :],
                        in_=transP_v[:, jl:jl + 1, :T],
                        func=mybir.ActivationFunctionType.Identity,
                        bias=fsb[:, fo + jl:fo + jl + 1],
                        scale=1.0)
                bi = nc.gpsimd.tensor_tensor(
                    out=t_[:, 5:, :], in0=transP_v[:, 5:, :T],
                    in1=fsb[:, fo + 5:fo + JL].unsqueeze(2)
                    .broadcast_to([128, JL - 5, T]),
                    op=mybir.AluOpType.add)
                if gate[0] is not None:
                    add_dep_helper(bi.ins, gate[0].ins, sync=True,
                                   reason="pool after DVE mult")
                ftF[ss] = t_

            def build_B(ss):
                t_ = st.tile([128, JL, CB], f16, tag="ftB%d" % (ss % 2))
                bsb = featsB_sbs[(ss - 1) // 32]
                bo = ((ss - 1) % 32) * T
                bi = nc.gpsimd.tensor_tensor(
                    out=t_[:, :, :T], in0=transPB_v[:, :, :],
                    in1=bsb[:, bo:bo + T].unsqueeze(1)
                    .broadcast_to([128, JL, T]),
                    op=mybir.AluOpType.add)
                if gate[0] is not None:
                    add_dep_helper(bi.ins, gate[0].ins, sync=True,
                                   reason="pool after DVE mult")
                isb = injA_sbs[(ss - 1) // 32]
                io = ((ss - 1) % 32) * JL
                nc.gpsimd.tensor_copy(
                    out=t_[:, :, T:CB],
                    in_=isb[:, io:io + JL].unsqueeze(2))
                ftB[ss] = t_

            def ftmm_F(ss):
                c_ = pp.tile([128, JL, T], f32, tag="CF%d" % (ss % 2))
                nc.tensor.matmul(c_[:, :, :], i128_sb[:, :],
                                 ftF.pop(ss)[:, :, :], start=True, stop=True)
                cF[ss] = c_

            def ftmm_B(ss):
                c_ = pp.tile([128, JL, CB], f32, tag="CB%d" % (ss % 2))
                nc.tensor.matmul(c_[:, :, :], i128_sb[:, :],
                                 ftB.pop(ss)[:, :, :], start=True, stop=True)
                cB[ss] = c_

            for ss in range(1, 4):
                build_F(ss)
                build_B(ss)
            ftmm_F(1)
            ftmm_B(1)

            for s in range(1, NSB + 1):
                run_f = s <= NSF
                # next FT matmuls first in PE order
                if s + 1 <= NSF:
                    ftmm_F(s + 1)
                if s + 1 <= NSB:
                    ftmm_B(s + 1)
                # REP matmuls
                if run_f:
                    c_f = cF.pop(s)
                    nc.tensor.matmul(
                        c_f[:, :, :], rep_sb[:, :],
                        mfwF[:, :T].unsqueeze(1).broadcast_to([128, JL, T]),
                        start=False, stop=True, skip_group_check=True)
                c_b = cB.pop(s)
                nc.tensor.matmul(
                    c_b[:, :, :T], rep_sb[:, :],
                    mfwB[:, :T].unsqueeze(1).broadcast_to([128, JL, T]),
                    start=False, stop=True, skip_group_check=True)

                # DVE: fwd reduce+mult, then bwd reduce+clamped mult
                if run_f:
                    m_f = hist_sb[:, (s - 1) * JL: s * JL]
                    nc.vector.tensor_reduce(
                        m_f, c_f[:, :, :],
                        axis=mybir.AxisListType.X, op=mybir.AluOpType.max)
                    mfwF = st.tile([128, TP], f16, tag="mfwF")
                    nc.vector.tensor_tensor(
                        out=mfwF[:, :].rearrange("p (a b) -> p a b", a=JG),
                        in0=m_f.unsqueeze(1).broadcast_to([128, JG, JL]),
                        in1=g_sb[:, :, :], op=mybir.AluOpType.mult)
                m_b = histB_sb[:, (s - 1) * JL: s * JL]
                nc.vector.tensor_reduce(
                    m_b, c_b[:, :, :],
                    axis=mybir.AxisListType.X, op=mybir.AluOpType.max)
                mfwB = st.tile([128, TP], f16, tag="mfwB")
                mult_b = nc.vector.scalar_tensor_tensor(
                    out=mfwB[:, :].rearrange("p (a b) -> p a b", a=JG),
                    in0=m_b.unsqueeze(1).broadcast_to([128, JG, JL]),
                    scalar=clampB_sb[:, s - 1:s],
                    in1=g_sb[:, :, :],
                    op0=mybir.AluOpType.min, op1=mybir.AluOpType.mult)
                gate[0] = mult_b
                if s + 3 <= NSF:
                    build_F(s + 3)
                if s + 3 <= NSB:
                    build_B(s + 3)

                if s % 64 == 0 and s < NSB:
                    lo, hi = (s - 64) * JL, s * JL
                    if s <= NSF:
                        nc.sync.dma_start(out=hist_d[:, lo:hi],
                                          in_=hist_sb[:, lo:hi])
                    nc.sync.dma_start(out=histB_d[:, lo:hi],
                                      in_=histB_sb[:, lo:hi])

            done = 192 * JL
            nc.sync.dma_start(out=hist_d[:, done:NSF * JL],
                              in_=hist_sb[:, done:NSF * JL])
            nc.sync.dma_start(out=histB_d[:, done:NSB * JL],
                              in_=histB_sb[:, done:NSB * JL])

    nc.compile()
    return nc


def _unpack(hist, nsteps):
    h = hist.astype(np.float32).reshape(JG, BL, nsteps, JL).transpose(2, 1, 0, 3)
    return h.reshape(nsteps, BL, TP)[:, :, :T]


def kernel(feats, mask, transitions):
    from concourse.bass_utils import run_bass_kernel_spmd

    feats = np.asarray(feats, dtype=np.float32)
    mask_np = np.asarray(mask).astype(bool)
    trans = np.asarray(transitions, dtype=np.float32)

    per_core = _host_prep2(feats, mask_np, trans)
    nc = build_bass()
    res = run_bass_kernel_spmd(nc, per_core, core_ids=list(range(NCORES)))

    c = feats.max(axis=2)
    lengths = mask_np.astype(np.int64).sum(axis=1)
    lp = lengths - 1
    bidx = np.arange(B)

    # assemble fwd part' (t=0..MID-1) and bwd beta' (t=MID-1..S-1)
    fwd = np.empty((MID, B, T), dtype=np.float32)
    fwd[0] = feats[:, 0, :] + trans[START][None, :] - c[:, 0:1]
    beta = np.empty((S, B, T), dtype=np.float32)
    trE = trans[:, END]
    beta[S - 1] = np.where((lp == S - 1)[:, None], trE[None, :], NEG)
    for ci in range(NCORES):
        sl = slice(ci * BL, (ci + 1) * BL)
        fwd[1:, sl] = _unpack(res.results[ci]["hist"], NSF)
        hb = _unpack(res.results[ci]["histB"], NSB)          # s=1..256
        beta[MID - 1:S - 1, sl] = hb[::-1]                    # t=255..510

    mid_tag = np.argmax(fwd[MID - 1] + beta[MID - 1], axis=1).astype(np.int32)

    decode = np.zeros((S, B), dtype=np.int32)
    decode[MID - 1] = mid_tag
    ptr = mid_tag.copy()
    trT = np.ascontiguousarray(trans.T)
    for t in range(MID - 2, -1, -1):
        sc = feats[bidx, t + 1, ptr][:, None] + trT[ptr]
        bp = np.argmax(sc + fwd[t], axis=1).astype(np.int32)
        decode[t] = bp
        ptr = bp
    tag = mid_tag.copy()
    final_tag = np.where(lp == MID - 1, mid_tag, 0).astype(np.int32)
    for t in range(MID, S):
        cur = trans[tag, :] + feats[bidx, t, :] + beta[t]
        nxt = np.argmax(cur, axis=1).astype(np.int32)
        active = t <= lp
        tag = np.where(active, nxt, tag).astype(np.int32)
        final_tag = np.where(active & (lp == t), tag, final_tag)
        decode[t] = np.where(active, tag, 0)
    decode[S - 1] = np.where(lp == S - 1, decode[S - 1], final_tag)
    return decode.T.astype(np.int32)



# revision 29
# speedup vs baseline: 1.0042x; 1.0042x over previous
"""Batched CRF Viterbi decode on 8 TRN2 NeuronCores.

Data-parallel over batch (16 sequences per core). The 511-step sequential
max-plus recurrence is split into TWO independent 255/256-step chains that
run concurrently on each core: a forward chain (part_t for t=1..255) and a
backward chain (beta_t for t=510..255, beta = best tail score including the
final ->END hop). All sequence lengths are >= 256, so every sequence end
falls in the backward half; ends are handled by an inject column (51st
column of the backward score tile, = trans[i,END] at t==last_pos) and a
per-(b,t) clamp (scalar_tensor_tensor min) that pins the state to NEG on
steps past the sequence end. The host stitches the halves at t=255 via
argmax(part+beta) and reconstructs the reference-equivalent decode.

Both chains recenter per (b,t) by c=max_j feats[b,t,j] (host-folded), so
states stay small enough to travel through the PE as float16 (1 cycle/row).
Per chain per step: Pool builds ft16 in SBUF; an fp16 identity matmul moves
it to PSUM (issued first in PE order so reduce sem-waits stay precise); one
fp16 K=128 matmul (REP128 @ zero-embedded state) accumulates the gathered
state; DVE reduces max over the score axis into the fp32 history and
re-embeds the fp16 state (backward: with the clamp via stt).
"""

import numpy as np

B, S, T = 128, 512, 50
NCORES = 8
BL = B // NCORES          # 16 sequences per core
JG, JL = 8, 7             # 8 groups x 7 tags = 56 padded tags
TP = JG * JL              # 56
NF = JL * TP              # fwd transP columns
MID = S // 2              # 256
NF_F = MID * JL           # fwd feats cols (t=0..255)
NSF = MID - 1             # 255 fwd steps (t=1..255)
NSB = MID                 # 256 bwd steps (t=510..255 plus seed t=511)
CB = T + 1                # bwd score cols: 50 + inject
START, END = T - 2, T - 1
NEG = np.float32(-25000.0)  # padding; must survive float16
BIG = np.float32(60000.0)


def _host_prep2(feats, mask, transitions):
    """Build per-core device input arrays for both chains."""
    f = np.ascontiguousarray(feats, dtype=np.float32)         # (B,S,T)
    tr = np.ascontiguousarray(transitions, dtype=np.float32)  # (T,T)
    c = f.max(axis=2)                                         # (B,S)
    lengths = mask.astype(np.int64).sum(axis=1)
    lp = lengths - 1                                          # in [255,511]

    trp = np.full((TP, TP), NEG, dtype=np.float32)
    trp[:T, :T] = tr

    k = np.arange(128)
    # fwd: transP[p=(jg,b), (jl, i)] = trp[i, jg*7+jl]
    transP = np.empty((128, JL, TP), dtype=np.float32)
    for g in range(JG):
        transP[g * BL:(g + 1) * BL] = trp[:, g * JL:(g + 1) * JL].T[None]
    transP = np.ascontiguousarray(transP.reshape(128, NF))
    # bwd: transPB[p=(ig,b), (il, j)] = trp[ig*7+il, j]  (j = 0..49)
    transPB = np.empty((128, JL, T), dtype=np.float32)
    for g in range(JG):
        transPB[g * BL:(g + 1) * BL] = trp[g * JL:(g + 1) * JL, :T][None]
    transPB = np.ascontiguousarray(transPB.reshape(128, JL * T))

    REP128 = (k[:, None] % BL == k[None, :] % BL).astype(np.float16)
    G = ((np.arange(TP)[None, :] // JL) == (k[:, None] // BL)).astype(np.float32)

    # bwd recentering: cb[b,tau] = c for real steps, 0 for masked
    cb = np.where(np.arange(S)[None, :] <= lp[:, None], c, 0.0).astype(np.float32)

    per_core = []
    for ci in range(NCORES):
        sl = slice(ci * BL, (ci + 1) * BL)
        fb, cbf, cbb, lpb = f[sl], c[sl], cb[sl], lp[sl]      # per-core views

        # ---- forward arrays (t = 0..MID-1) ----
        fp = np.zeros((BL, MID, TP), dtype=np.float32)
        fp[:, :, :T] = fb[:, :MID, :] - cbf[:, :MID, None]
        fa = fp.reshape(BL, MID, JG, JL).transpose(2, 0, 1, 3).reshape(128, NF_F)
        part0 = np.full((BL, TP), NEG, dtype=np.float32)
        part0[:, :T] = fb[:, 0, :] + tr[START][None, :] - cbf[:, 0:1]
        mw0 = np.repeat(part0[None], JG, axis=0).reshape(128, TP) * G

        # ---- backward arrays: step s=1..NSB computes t = S-1-s ----
        svec = np.arange(1, NSB + 1)
        tvec = S - 1 - svec                                   # 510..255
        # featsB[p=(ig,b), (s-1)*T + j] = feats[b, t(s)+1, j] - cb[b, t(s)+1]
        fB = (fb[:, tvec + 1, :] - cbb[np.arange(BL)[:, None], tvec + 1][:, :, None])
        featsB = np.broadcast_to(fB[None], (JG, BL, NSB, T)) \
            .reshape(128, NSB * T).astype(np.float16)
        # injA[p=(ig,b), (s-1)*7+il] = trans[i(ig,il),END] if t(s)==lp else NEG
        trE = np.full(TP, NEG, dtype=np.float32)
        trE[:T] = tr[:, END]
        hit = (tvec[None, :] == lpb[:, None])                 # (BL, NSB)
        injA = np.where(
            hit[None, :, :, None],                            # (1,BL,NSB,1)
            trE.reshape(JG, 1, 1, JL),                        # (JG,1,1,JL)
            NEG).transpose(0, 1, 2, 3).reshape(JG * BL, NSB * JL)
        injA = np.ascontiguousarray(injA.astype(np.float16))
        # clampB[p, s-1] = NEG if t(s) > lp else BIG
        clampB = np.where((tvec[None, :] > lpb[:, None])[None],
                          NEG, BIG)
        clampB = np.broadcast_to(clampB, (JG, BL, NSB)) \
            .reshape(128, NSB).astype(np.float32)
        # seed beta'_{511}
        seed = np.where((lpb == S - 1)[:, None], trE[None, :T], NEG)
        seedp = np.full((BL, TP), NEG, dtype=np.float32)
        seedp[:, :T] = seed
        mw0B = np.repeat(seedp[None], JG, axis=0).reshape(128, TP) * G

        per_core.append({
            "feats_arr": np.ascontiguousarray(fa.astype(np.float16)),
            "transP": transP,
            "transPB": transPB,
            "featsB": featsB,
            "injA": injA,
            "clampB": np.ascontiguousarray(clampB),
            "I128": np.eye(128, dtype=np.float16),
            "REP128": REP128,
            "G": G,
            "mw0": np.ascontiguousarray(mw0.astype(np.float16)),
            "mw0B": np.ascontiguousarray(mw0B.astype(np.float16)),
        })
    return per_core


def build_bass():
    import concourse.bacc as bacc
    import concourse.mybir as mybir
    import concourse.tile as tile

    f32 = mybir.dt.float32
    f16 = mybir.dt.float16
    nc = bacc.Bacc("TRN2", target_bir_lowering=False, debug=False,
                   num_devices=NCORES)

    feats_d = nc.declare_dram_parameter("feats_arr", [128, NF_F], f16, isOutput=False)
    transP_d = nc.declare_dram_parameter("transP", [128, NF], f32, isOutput=False)
    transPB_d = nc.declare_dram_parameter("transPB", [128, JL * T], f32, isOutput=False)
    featsB_d = nc.declare_dram_parameter("featsB", [128, NSB * T], f16, isOutput=False)
    injA_d = nc.declare_dram_parameter("injA", [128, NSB * JL], f16, isOutput=False)
    clampB_d = nc.declare_dram_parameter("clampB", [128, NSB], f32, isOutput=False)
    i128_d = nc.declare_dram_parameter("I128", [128, 128], f16, isOutput=False)
    rep_d = nc.declare_dram_parameter("REP128", [128, 128], f16, isOutput=False)
    g_d = nc.declare_dram_parameter("G", [128, TP], f32, isOutput=False)
    mw0_d = nc.declare_dram_parameter("mw0", [128, TP], f16, isOutput=False)
    mw0B_d = nc.declare_dram_parameter("mw0B", [128, TP], f16, isOutput=False)
    hist_d = nc.declare_dram_parameter("hist", [128, NSF * JL], f16, isOutput=True)
    histB_d = nc.declare_dram_parameter("histB", [128, NSB * JL], f16, isOutput=True)

    with tile.TileContext(nc) as tc:
        with (
            tc.tile_pool(name="static", bufs=1) as sp,
            tc.tile_pool(name="state", bufs=6) as st,
            tc.tile_pool(name="psum", bufs=2, space="PSUM") as pp,
        ):
            def load(handle, shape, dt, tag):
                t_ = sp.tile(shape, dt, tag=tag)
                nc.sync.dma_start(out=t_[:, :], in_=handle[:, :])
                return t_
            feats_sb = sp.tile([128, NF_F], f16)
            featsB_sb = sp.tile([128, NSB * T], f16)
            fchunk = NF_F // 8
            bchunk = NSB * T // 8
            for ci8 in range(8):
                nc.sync.dma_start(
                    out=featsB_sb[:, ci8 * bchunk:(ci8 + 1) * bchunk],
                    in_=featsB_d[:, ci8 * bchunk:(ci8 + 1) * bchunk])
                nc.sync.dma_start(
                    out=feats_sb[:, ci8 * fchunk:(ci8 + 1) * fchunk],
                    in_=feats_d[:, ci8 * fchunk:(ci8 + 1) * fchunk])
            transP_sb = load(transP_d, [128, NF], f32, "trP")
            transPB_sb = load(transPB_d, [128, JL * T], f32, "trPB")
            injA_sb = load(injA_d, [128, NSB * JL], f16, "injA")
            clampB_sb = load(clampB_d, [128, NSB], f32, "clampB")
            i128_sb = load(i128_d, [128, 128], f16, "i128")
            rep_sb = load(rep_d, [128, 128], f16, "rep")
            g_sb = sp.tile([128, JG, JL], f32)
            nc.sync.dma_start(out=g_sb[:, :, :], in_=g_d[:, :].rearrange(
                "p (a b) -> p a b", a=JG))

            hist_sb = sp.tile([128, NSF * JL], f16)
            histB_sb = sp.tile([128, NSB * JL], f16)

            mfwF = st.tile([128, TP], f16, tag="mfwF")
            nc.sync.dma_start(out=mfwF[:, :], in_=mw0_d[:, :])
            mfwB = st.tile([128, TP], f16, tag="mfwB")
            nc.sync.dma_start(out=mfwB[:, :], in_=mw0B_d[:, :])

            transP_v = transP_sb[:, :].rearrange("p (a b) -> p a b", a=JL)
            transPB_v = transPB_sb[:, :].rearrange("p (a b) -> p a b", a=JL)

            from concourse.tile_rust import add_dep_helper

            ftF, ftB, cF, cB = {}, {}, {}, {}
            gate = [None]

            def build_F(ss):
                # fwd ft build split across the idle Act engine (4 rows via
                # Copy-activation with per-partition feats bias) and Pool
                # (3 rows), relieving the Pool bottleneck
                t_ = st.tile([128, JL, T], f16, tag="ftF%d" % (ss % 2))
                for jl in range(5):
                    nc.scalar.activation(
                        out=t_[:, jl:jl + 1, :],
                        in_=transP_v[:, jl:jl + 1, :T],
                        func=mybir.ActivationFunctionType.Identity,
                        bias=feats_sb[:, ss * JL + jl:ss * JL + jl + 1],
                        scale=1.0)
                bi = nc.gpsimd.tensor_tensor(
                    out=t_[:, 5:, :], in0=transP_v[:, 5:, :T],
                    in1=feats_sb[:, ss * JL + 5:(ss + 1) * JL].unsqueeze(2)
                    .broadcast_to([128, JL - 5, T]),
                    op=mybir.AluOpType.add)
                if gate[0] is not None:
                    add_dep_helper(bi.ins, gate[0].ins, sync=True,
                                   reason="pool after DVE mult")
                ftF[ss] = t_

            def build_B(ss):
                t_ = st.tile([128, JL, CB], f16, tag="ftB%d" % (ss % 2))
                bi = nc.gpsimd.tensor_tensor(
                    out=t_[:, :, :T], in0=transPB_v[:, :, :],
                    in1=featsB_sb[:, (ss - 1) * T:ss * T].unsqueeze(1)
                    .broadcast_to([128, JL, T]),
                    op=mybir.AluOpType.add)
                if gate[0] is not None:
                    add_dep_helper(bi.ins, gate[0].ins, sync=True,
                                   reason="pool after DVE mult")
                nc.gpsimd.tensor_copy(
                    out=t_[:, :, T:CB],
                    in_=injA_sb[:, (ss - 1) * JL:ss * JL].unsqueeze(2))
                ftB[ss] = t_

            def ftmm_F(ss):
                c_ = pp.tile([128, JL, T], f32, tag="CF%d" % (ss % 2))
                nc.tensor.matmul(c_[:, :, :], i128_sb[:, :],
                                 ftF.pop(ss)[:, :, :], start=True, stop=True)
                cF[ss] = c_

            def ftmm_B(ss):
                c_ = pp.tile([128, JL, CB], f32, tag="CB%d" % (ss % 2))
                nc.tensor.matmul(c_[:, :, :], i128_sb[:, :],
                                 ftB.pop(ss)[:, :, :], start=True, stop=True)
                cB[ss] = c_

            for ss in range(1, 4):
                build_F(ss)
                build_B(ss)
            ftmm_F(1)
            ftmm_B(1)

            for s in range(1, NSB + 1):
                run_f = s <= NSF
                # next FT matmuls first in PE order
                if s + 1 <= NSF:
                    ftmm_F(s + 1)
                if s + 1 <= NSB:
                    ftmm_B(s + 1)
                # REP matmuls
                if run_f:
                    c_f = cF.pop(s)
                    nc.tensor.matmul(
                        c_f[:, :, :], rep_sb[:, :],
                        mfwF[:, :T].unsqueeze(1).broadcast_to([128, JL, T]),
                        start=False, stop=True, skip_group_check=True)
                c_b = cB.pop(s)
                nc.tensor.matmul(
                    c_b[:, :, :T], rep_sb[:, :],
                    mfwB[:, :T].unsqueeze(1).broadcast_to([128, JL, T]),
                    start=False, stop=True, skip_group_check=True)

                # DVE: fwd reduce+mult, then bwd reduce+clamped mult
                if run_f:
                    m_f = hist_sb[:, (s - 1) * JL: s * JL]
                    nc.vector.tensor_reduce(
                        m_f, c_f[:, :, :],
                        axis=mybir.AxisListType.X, op=mybir.AluOpType.max)
                    mfwF = st.tile([128, TP], f16, tag="mfwF")
                    nc.vector.tensor_tensor(
                        out=mfwF[:, :].rearrange("p (a b) -> p a b", a=JG),
                        in0=m_f.unsqueeze(1).broadcast_to([128, JG, JL]),
                        in1=g_sb[:, :, :], op=mybir.AluOpType.mult)
                m_b = histB_sb[:, (s - 1) * JL: s * JL]
                nc.vector.tensor_reduce(
                    m_b, c_b[:, :, :],
                    axis=mybir.AxisListType.X, op=mybir.AluOpType.max)
                mfwB = st.tile([128, TP], f16, tag="mfwB")
                mult_b = nc.vector.scalar_tensor_tensor(
                    out=mfwB[:, :].rearrange("p (a b) -> p a b", a=JG),
                    in0=m_b.unsqueeze(1).broadcast_to([128, JG, JL]),
                    scalar=clampB_sb[:, s - 1:s],
                    in1=g_sb[:, :, :],
                    op0=mybir.AluOpType.min, op1=mybir.AluOpType.mult)
                gate[0] = mult_b
                if s + 3 <= NSF:
                    build_F(s + 3)
                if s + 3 <= NSB:
                    build_B(s + 3)

                if s % 64 == 0 and s < NSB:
                    lo, hi = (s - 64) * JL, s * JL
                    if s <= NSF:
                        nc.sync.dma_start(out=hist_d[:, lo:hi],
                                          in_=hist_sb[:, lo:hi])
                    nc.sync.dma_start(out=histB_d[:, lo:hi],
                                      in_=histB_sb[:, lo:hi])

            done = 192 * JL
            nc.sync.dma_start(out=hist_d[:, done:NSF * JL],
                              in_=hist_sb[:, done:NSF * JL])
            nc.sync.dma_start(out=histB_d[:, done:NSB * JL],
                              in_=histB_sb[:, done:NSB * JL])

    nc.compile()
    return nc


def _unpack(hist, nsteps):
    h = hist.astype(np.float32).reshape(JG, BL, nsteps, JL).transpose(2, 1, 0, 3)
    return h.reshape(nsteps, BL, TP)[:, :, :T]


def kernel(feats, mask, transitions):
    from concourse.bass_utils import run_bass_kernel_spmd

    feats = np.asarray(feats, dtype=np.float32)
    mask_np = np.asarray(mask).astype(bool)
    trans = np.asarray(transitions, dtype=np.float32)

    per_core = _host_prep2(feats, mask_np, trans)
    nc = build_bass()
    res = run_bass_kernel_spmd(nc, per_core, core_ids=list(range(NCORES)))

    c = feats.max(axis=2)
    lengths = mask_np.astype(np.int64).sum(axis=1)
    lp = lengths - 1
    bidx = np.arange(B)

    # assemble fwd part' (t=0..MID-1) and bwd beta' (t=MID-1..S-1)
    fwd = np.empty((MID, B, T), dtype=np.float32)
    fwd[0] = feats[:, 0, :] + trans[START][None, :] - c[:, 0:1]
    beta = np.empty((S, B, T), dtype=np.float32)
    trE = trans[:, END]
    beta[S - 1] = np.where((lp == S - 1)[:, None], trE[None, :], NEG)
    for ci in range(NCORES):
        sl = slice(ci * BL, (ci + 1) * BL)
        fwd[1:, sl] = _unpack(res.results[ci]["hist"], NSF)
        hb = _unpack(res.results[ci]["histB"], NSB)          # s=1..256
        beta[MID - 1:S - 1, sl] = hb[::-1]                    # t=255..510

    mid_tag = np.argmax(fwd[MID - 1] + beta[MID - 1], axis=1).astype(np.int32)

    decode = np.zeros((S, B), dtype=np.int32)
    decode[MID - 1] = mid_tag
    ptr = mid_tag.copy()
    trT = np.ascontiguousarray(trans.T)
    for t in range(MID - 2, -1, -1):
        sc = feats[bidx, t + 1, ptr][:, None] + trT[ptr]
        bp = np.argmax(sc + fwd[t], axis=1).astype(np.int32)
        decode[t] = bp
        ptr = bp
    tag = mid_tag.copy()
    final_tag = np.where(lp == MID - 1, mid_tag, 0).astype(np.int32)
    for t in range(MID, S):
        cur = trans[tag, :] + feats[bidx, t, :] + beta[t]
        nxt = np.argmax(cur, axis=1).astype(np.int32)
        active = t <= lp
        tag = np.where(active, nxt, tag).astype(np.int32)
        final_tag = np.where(active & (lp == t), tag, final_tag)
        decode[t] = np.where(active, tag, 0)
    decode[S - 1] = np.where(lp == S - 1, decode[S - 1], final_tag)
    return decode.T.astype(np.int32)



# revision 30
# speedup vs baseline: 1.0049x; 1.0007x over previous
"""Batched CRF Viterbi decode on 8 TRN2 NeuronCores.

Data-parallel over batch (16 sequences per core). The 511-step sequential
max-plus recurrence is split into TWO independent 255/256-step chains that
run concurrently on each core: a forward chain (part_t for t=1..255) and a
backward chain (beta_t for t=510..255, beta = best tail score including the
final ->END hop). All sequence lengths are >= 256, so every sequence end
falls in the backward half; ends are handled by an inject column (51st
column of the backward score tile, = trans[i,END] at t==last_pos) and a
per-(b,t) clamp (scalar_tensor_tensor min) that pins the state to NEG on
steps past the sequence end. The host stitches the halves at t=255 via
argmax(part+beta) and reconstructs the reference-equivalent decode.

Both chains recenter per (b,t) by c=max_j feats[b,t,j] (host-folded), so
states stay small enough to travel through the PE as float16 (1 cycle/row).
Per chain per step: Pool builds ft16 in SBUF; an fp16 identity matmul moves
it to PSUM (issued first in PE order so reduce sem-waits stay precise); one
fp16 K=128 matmul (REP128 @ zero-embedded state) accumulates the gathered
state; DVE reduces max over the score axis into the fp32 history and
re-embeds the fp16 state (backward: with the clamp via stt).
"""

import numpy as np

B, S, T = 128, 512, 50
NCORES = 8
BL = B // NCORES          # 16 sequences per core
JG, JL = 8, 7             # 8 groups x 7 tags = 56 padded tags
TP = JG * JL              # 56
NF = JL * TP              # fwd transP columns
MID = S // 2              # 256
NF_F = MID * JL           # fwd feats cols (t=0..255)
NSF = MID - 1             # 255 fwd steps (t=1..255)
NSB = MID                 # 256 bwd steps (t=510..255 plus seed t=511)
CB = T + 1                # bwd score cols: 50 + inject
START, END = T - 2, T - 1
NEG = np.float32(-25000.0)  # padding; must survive float16
BIG = np.float32(60000.0)


def _host_prep2(feats, mask, transitions):
    """Build per-core device input arrays for both chains."""
    f = np.ascontiguousarray(feats, dtype=np.float32)         # (B,S,T)
    tr = np.ascontiguousarray(transitions, dtype=np.float32)  # (T,T)
    c = f.max(axis=2)                                         # (B,S)
    lengths = mask.astype(np.int64).sum(axis=1)
    lp = lengths - 1                                          # in [255,511]

    trp = np.full((TP, TP), NEG, dtype=np.float32)
    trp[:T, :T] = tr

    k = np.arange(128)
    # fwd: transP[p=(jg,b), (jl, i)] = trp[i, jg*7+jl]
    transP = np.empty((128, JL, TP), dtype=np.float32)
    for g in range(JG):
        transP[g * BL:(g + 1) * BL] = trp[:, g * JL:(g + 1) * JL].T[None]
    transP = np.ascontiguousarray(transP.reshape(128, NF))
    # bwd: transPB[p=(ig,b), (il, j)] = trp[ig*7+il, j]  (j = 0..49)
    transPB = np.empty((128, JL, T), dtype=np.float32)
    for g in range(JG):
        transPB[g * BL:(g + 1) * BL] = trp[g * JL:(g + 1) * JL, :T][None]
    transPB = np.ascontiguousarray(transPB.reshape(128, JL * T))

    REP128 = (k[:, None] % BL == k[None, :] % BL).astype(np.float16)
    G = ((np.arange(TP)[None, :] // JL) == (k[:, None] // BL)).astype(np.float32)

    # bwd recentering: cb[b,tau] = c for real steps, 0 for masked
    cb = np.where(np.arange(S)[None, :] <= lp[:, None], c, 0.0).astype(np.float32)

    per_core = []
    for ci in range(NCORES):
        sl = slice(ci * BL, (ci + 1) * BL)
        fb, cbf, cbb, lpb = f[sl], c[sl], cb[sl], lp[sl]      # per-core views

        # ---- forward arrays (t = 0..MID-1) ----
        fp = np.zeros((BL, MID, TP), dtype=np.float32)
        fp[:, :, :T] = fb[:, :MID, :] - cbf[:, :MID, None]
        fa = fp.reshape(BL, MID, JG, JL).transpose(2, 0, 1, 3).reshape(128, NF_F)
        part0 = np.full((BL, TP), NEG, dtype=np.float32)
        part0[:, :T] = fb[:, 0, :] + tr[START][None, :] - cbf[:, 0:1]
        mw0 = np.repeat(part0[None], JG, axis=0).reshape(128, TP) * G

        # ---- backward arrays: step s=1..NSB computes t = S-1-s ----
        svec = np.arange(1, NSB + 1)
        tvec = S - 1 - svec                                   # 510..255
        # featsB[p=(ig,b), (s-1)*T + j] = feats[b, t(s)+1, j] - cb[b, t(s)+1]
        fB = (fb[:, tvec + 1, :] - cbb[np.arange(BL)[:, None], tvec + 1][:, :, None])
        featsB = np.broadcast_to(fB[None], (JG, BL, NSB, T)) \
            .reshape(128, NSB * T).astype(np.float16)
        # injA[p=(ig,b), (s-1)*7+il] = trans[i(ig,il),END] if t(s)==lp else NEG
        trE = np.full(TP, NEG, dtype=np.float32)
        trE[:T] = tr[:, END]
        hit = (tvec[None, :] == lpb[:, None])                 # (BL, NSB)
        injA = np.where(
            hit[None, :, :, None],                            # (1,BL,NSB,1)
            trE.reshape(JG, 1, 1, JL),                        # (JG,1,1,JL)
            NEG).transpose(0, 1, 2, 3).reshape(JG * BL, NSB * JL)
        injA = np.ascontiguousarray(injA.astype(np.float16))
        # clampB[p, s-1] = NEG if t(s) > lp else BIG
        clampB = np.where((tvec[None, :] > lpb[:, None])[None],
                          NEG, BIG)
        clampB = np.broadcast_to(clampB, (JG, BL, NSB)) \
            .reshape(128, NSB).astype(np.float32)
        # seed beta'_{511}
        seed = np.where((lpb == S - 1)[:, None], trE[None, :T], NEG)
        seedp = np.full((BL, TP), NEG, dtype=np.float32)
        seedp[:, :T] = seed
        mw0B = np.repeat(seedp[None], JG, axis=0).reshape(128, TP) * G

        per_core.append({
            "feats_arr": np.ascontiguousarray(fa.astype(np.float16)),
            "transP": transP,
            "transPB": transPB,
            "featsB": featsB,
            "injA": injA,
            "clampB": np.ascontiguousarray(clampB),
            "I128": np.eye(128, dtype=np.float16),
            "REP128": REP128,
            "G": G,
            "mw0": np.ascontiguousarray(mw0.astype(np.float16)),
            "mw0B": np.ascontiguousarray(mw0B.astype(np.float16)),
        })
    return per_core


def build_bass():
    import concourse.bacc as bacc
    import concourse.mybir as mybir
    import concourse.tile as tile

    f32 = mybir.dt.float32
    f16 = mybir.dt.float16
    nc = bacc.Bacc("TRN2", target_bir_lowering=False, debug=False,
                   num_devices=NCORES)

    feats_d = nc.declare_dram_parameter("feats_arr", [128, NF_F], f16, isOutput=False)
    transP_d = nc.declare_dram_parameter("transP", [128, NF], f32, isOutput=False)
    transPB_d = nc.declare_dram_parameter("transPB", [128, JL * T], f32, isOutput=False)
    featsB_d = nc.declare_dram_parameter("featsB", [128, NSB * T], f16, isOutput=False)
    injA_d = nc.declare_dram_parameter("injA", [128, NSB * JL], f16, isOutput=False)
    clampB_d = nc.declare_dram_parameter("clampB", [128, NSB], f32, isOutput=False)
    i128_d = nc.declare_dram_parameter("I128", [128, 128], f16, isOutput=False)
    rep_d = nc.declare_dram_parameter("REP128", [128, 128], f16, isOutput=False)
    g_d = nc.declare_dram_parameter("G", [128, TP], f32, isOutput=False)
    mw0_d = nc.declare_dram_parameter("mw0", [128, TP], f16, isOutput=False)
    mw0B_d = nc.declare_dram_parameter("mw0B", [128, TP], f16, isOutput=False)
    hist_d = nc.declare_dram_parameter("hist", [128, NSF * JL], f16, isOutput=True)
    histB_d = nc.declare_dram_parameter("histB", [128, NSB * JL], f16, isOutput=True)

    with tile.TileContext(nc) as tc:
        with (
            tc.tile_pool(name="static", bufs=1) as sp,
            tc.tile_pool(name="state", bufs=6) as st,
            tc.tile_pool(name="psum", bufs=2, space="PSUM") as pp,
        ):
            def load(handle, shape, dt, tag):
                t_ = sp.tile(shape, dt, tag=tag)
                nc.sync.dma_start(out=t_[:, :], in_=handle[:, :])
                return t_
            feats_sb = sp.tile([128, NF_F], f16)
            featsB_sb = sp.tile([128, NSB * T], f16)
            fchunk = NF_F // 8
            bchunk = NSB * T // 8
            for ci8 in range(8):
                nc.sync.dma_start(
                    out=featsB_sb[:, ci8 * bchunk:(ci8 + 1) * bchunk],
                    in_=featsB_d[:, ci8 * bchunk:(ci8 + 1) * bchunk])
                nc.sync.dma_start(
                    out=feats_sb[:, ci8 * fchunk:(ci8 + 1) * fchunk],
                    in_=feats_d[:, ci8 * fchunk:(ci8 + 1) * fchunk])
            transP_sb = load(transP_d, [128, NF], f32, "trP")
            transPB_sb = load(transPB_d, [128, JL * T], f32, "trPB")
            injA_sb = load(injA_d, [128, NSB * JL], f16, "injA")
            clampB_sb = load(clampB_d, [128, NSB], f32, "clampB")
            i128_sb = load(i128_d, [128, 128], f16, "i128")
            rep_sb = load(rep_d, [128, 128], f16, "rep")
            g_sb = sp.tile([128, JG, JL], f32)
            nc.sync.dma_start(out=g_sb[:, :, :], in_=g_d[:, :].rearrange(
                "p (a b) -> p a b", a=JG))

            hist_sb = sp.tile([128, NSF * JL], f16)
            histB_sb = sp.tile([128, NSB * JL], f16)

            mfwF = st.tile([128, TP], f16, tag="mfwF")
            nc.sync.dma_start(out=mfwF[:, :], in_=mw0_d[:, :])
            mfwB = st.tile([128, TP], f16, tag="mfwB")
            nc.sync.dma_start(out=mfwB[:, :], in_=mw0B_d[:, :])

            transP_v = transP_sb[:, :].rearrange("p (a b) -> p a b", a=JL)
            transPB_v = transPB_sb[:, :].rearrange("p (a b) -> p a b", a=JL)

            from concourse.tile_rust import add_dep_helper

            ftF, ftB, cF, cB = {}, {}, {}, {}
            gate = [None]

            def build_F(ss):
                # fwd ft build split across the idle Act engine (4 rows via
                # Copy-activation with per-partition feats bias) and Pool
                # (3 rows), relieving the Pool bottleneck
                t_ = st.tile([128, JL, T], f16, tag="ftF%d" % (ss % 2))
                for jl in range(5):
                    nc.scalar.activation(
                        out=t_[:, jl:jl + 1, :],
                        in_=transP_v[:, jl:jl + 1, :T],
                        func=mybir.ActivationFunctionType.Identity,
                        bias=feats_sb[:, ss * JL + jl:ss * JL + jl + 1],
                        scale=1.0)
                bi = nc.gpsimd.tensor_tensor(
                    out=t_[:, 5:, :], in0=transP_v[:, 5:, :T],
                    in1=feats_sb[:, ss * JL + 5:(ss + 1) * JL].unsqueeze(2)
                    .broadcast_to([128, JL - 5, T]),
                    op=mybir.AluOpType.add)
                if gate[0] is not None:
                    add_dep_helper(bi.ins, gate[0].ins, sync=True,
                                   reason="pool after DVE mult")
                ftF[ss] = t_

            def build_B(ss):
                t_ = st.tile([128, JL, CB], f16, tag="ftB%d" % (ss % 2))
                bi = nc.gpsimd.tensor_tensor(
                    out=t_[:, :, :T], in0=transPB_v[:, :, :],
                    in1=featsB_sb[:, (ss - 1) * T:ss * T].unsqueeze(1)
                    .broadcast_to([128, JL, T]),
                    op=mybir.AluOpType.add)
                if gate[0] is not None:
                    add_dep_helper(bi.ins, gate[0].ins, sync=True,
                                   reason="pool after DVE mult")
                nc.gpsimd.tensor_copy(
                    out=t_[:, :, T:CB],
                    in_=injA_sb[:, (ss - 1) * JL:ss * JL].unsqueeze(2))
                ftB[ss] = t_

            def ftmm_F(ss):
                c_ = pp.tile([128, JL, T], f32, tag="CF%d" % (ss % 2))
                nc.tensor.matmul(c_[:, :, :], i128_sb[:, :],
                                 ftF.pop(ss)[:, :, :], start=True, stop=True)
                cF[ss] = c_

            def ftmm_B(ss):
                c_ = pp.tile([128, JL, CB], f32, tag="CB%d" % (ss % 2))
                nc.tensor.matmul(c_[:, :, :], i128_sb[:, :],
                                 ftB.pop(ss)[:, :, :], start=True, stop=True)
                cB[ss] = c_

            for ss in range(1, 4):
                build_F(ss)
                build_B(ss)
            ftmm_F(1)
            ftmm_B(1)

            for s in range(1, NSB + 1):
                run_f = s <= NSF
                # next FT matmuls first in PE order
                if s + 1 <= NSF:
                    ftmm_F(s + 1)
                if s + 1 <= NSB:
                    ftmm_B(s + 1)
                # REP matmuls
                if run_f:
                    c_f = cF.pop(s)
                    nc.tensor.matmul(
                        c_f[:, :, :], rep_sb[:, :],
                        mfwF[:, :T].unsqueeze(1).broadcast_to([128, JL, T]),
                        start=False, stop=True, skip_group_check=True)
                c_b = cB.pop(s)
                nc.tensor.matmul(
                    c_b[:, :, :T], rep_sb[:, :],
                    mfwB[:, :T].unsqueeze(1).broadcast_to([128, JL, T]),
                    start=False, stop=True, skip_group_check=True)

                # DVE: fwd reduce+mult, then bwd reduce+clamped mult
                if run_f:
                    m_f = hist_sb[:, (s - 1) * JL: s * JL]
                    nc.vector.tensor_reduce(
                        m_f, c_f[:, :, :],
                        axis=mybir.AxisListType.X, op=mybir.AluOpType.max)
                    mfwF = st.tile([128, TP], f16, tag="mfwF")
                    nc.vector.tensor_tensor(
                        out=mfwF[:, :].rearrange("p (a b) -> p a b", a=JG),
                        in0=m_f.unsqueeze(1).broadcast_to([128, JG, JL]),
                        in1=g_sb[:, :, :], op=mybir.AluOpType.mult)
                m_b = histB_sb[:, (s - 1) * JL: s * JL]
                nc.vector.tensor_reduce(
                    m_b, c_b[:, :, :],
                    axis=mybir.AxisListType.X, op=mybir.AluOpType.max)
                mfwB = st.tile([128, TP], f16, tag="mfwB")
                mult_b = nc.vector.scalar_tensor_tensor(
                    out=mfwB[:, :].rearrange("p (a b) -> p a b", a=JG),
                    in0=m_b.unsqueeze(1).broadcast_to([128, JG, JL]),
                    scalar=clampB_sb[:, s - 1:s],
                    in1=g_sb[:, :, :],
                    op0=mybir.AluOpType.min, op1=mybir.AluOpType.mult)
                gate[0] = mult_b
                if s + 3 <= NSF:
                    build_F(s + 3)
                if s + 3 <= NSB:
                    build_B(s + 3)

                if s % 16 == 0 and s < NSB:
                    lo, hi = (s - 16) * JL, s * JL
                    if s <= NSF:
                        nc.sync.dma_start(out=hist_d[:, lo:hi],
                                          in_=hist_sb[:, lo:hi])
                    nc.sync.dma_start(out=histB_d[:, lo:hi],
                                      in_=histB_sb[:, lo:hi])

            done = 240 * JL
            nc.sync.dma_start(out=hist_d[:, done:NSF * JL],
                              in_=hist_sb[:, done:NSF * JL])
            nc.sync.dma_start(out=histB_d[:, done:NSB * JL],
                              in_=histB_sb[:, done:NSB * JL])

    nc.compile()
    return nc


def _unpack(hist, nsteps):
    h = hist.astype(np.float32).reshape(JG, BL, nsteps, JL).transpose(2, 1, 0, 3)
    return h.reshape(nsteps, BL, TP)[:, :, :T]


def kernel(feats, mask, transitions):
    from concourse.bass_utils import run_bass_kernel_spmd

    feats = np.asarray(feats, dtype=np.float32)
    mask_np = np.asarray(mask).astype(bool)
    trans = np.asarray(transitions, dtype=np.float32)

    per_core = _host_prep2(feats, mask_np, trans)
    nc = build_bass()
    res = run_bass_kernel_spmd(nc, per_core, core_ids=list(range(NCORES)))

    c = feats.max(axis=2)
    lengths = mask_np.astype(np.int64).sum(axis=1)
    lp = lengths - 1
    bidx = np.arange(B)

    # assemble fwd part' (t=0..MID-1) and bwd beta' (t=MID-1..S-1)
    fwd = np.empty((MID, B, T), dtype=np.float32)
    fwd[0] = feats[:, 0, :] + trans[START][None, :] - c[:, 0:1]
    beta = np.empty((S, B, T), dtype=np.float32)
    trE = trans[:, END]
    beta[S - 1] = np.where((lp == S - 1)[:, None], trE[None, :], NEG)
    for ci in range(NCORES):
        sl = slice(ci * BL, (ci + 1) * BL)
        fwd[1:, sl] = _unpack(res.results[ci]["hist"], NSF)
        hb = _unpack(res.results[ci]["histB"], NSB)          # s=1..256
        beta[MID - 1:S - 1, sl] = hb[::-1]                    # t=255..510

    mid_tag = np.argmax(fwd[MID - 1] + beta[MID - 1], axis=1).astype(np.int32)

    decode = np.zeros((S, B), dtype=np.int32)
    decode[MID - 1] = mid_tag
    ptr = mid_tag.copy()
    trT = np.ascontiguousarray(trans.T)
    for t in range(MID - 2, -1, -1):
        sc = feats[bidx, t + 1, ptr][:, None] + trT[ptr]
        bp = np.argmax(sc + fwd[t], axis=1).astype(np.int32)
        decode[t] = bp
        ptr = bp
    tag = mid_tag.copy()
    final_tag = np.where(lp == MID - 1, mid_tag, 0).astype(np.int32)
    for t in range(MID, S):
        cur = trans[tag, :] + feats[bidx, t, :] + beta[t]
        nxt = np.argmax(cur, axis=1).astype(np.int32)
        active = t <= lp
        tag = np.where(active, nxt, tag).astype(np.int32)
        final_tag = np.where(active & (lp == t), tag, final_tag)
        decode[t] = np.where(active, tag, 0)
    decode[S - 1] = np.where(lp == S - 1, decode[S - 1], final_tag)
    return decode.T.astype(np.int32)



# revision 31
# speedup vs baseline: 1.0125x; 1.0076x over previous
"""Batched CRF Viterbi decode on 8 TRN2 NeuronCores.

Data-parallel over batch (16 sequences per core). The 511-step sequential
max-plus recurrence is split into TWO independent 255/256-step chains that
run concurrently on each core: a forward chain (part_t for t=1..255) and a
backward chain (beta_t for t=510..255, beta = best tail score including the
final ->END hop). All sequence lengths are >= 256, so every sequence end
falls in the backward half; ends are handled by an inject column (51st
column of the backward score tile, = trans[i,END] at t==last_pos) and a
per-(b,t) clamp (scalar_tensor_tensor min) that pins the state to NEG on
steps past the sequence end. The host stitches the halves at t=255 via
argmax(part+beta) and reconstructs the reference-equivalent decode.

Both chains recenter per (b,t) by c=max_j feats[b,t,j] (host-folded), so
states stay small enough to travel through the PE as float16 (1 cycle/row).
Per chain per step: Pool builds ft16 in SBUF; an fp16 identity matmul moves
it to PSUM (issued first in PE order so reduce sem-waits stay precise); one
fp16 K=128 matmul (REP128 @ zero-embedded state) accumulates the gathered
state; DVE reduces max over the score axis into the fp32 history and
re-embeds the fp16 state (backward: with the clamp via stt).
"""

import numpy as np

B, S, T = 128, 512, 50
NCORES = 8
BL = B // NCORES          # 16 sequences per core
JG, JL = 8, 7             # 8 groups x 7 tags = 56 padded tags
TP = JG * JL              # 56
NF = JL * TP              # fwd transP columns
MID = S // 2              # 256
NF_F = MID * JL           # fwd feats cols (t=0..255)
NSF = MID - 1             # 255 fwd steps (t=1..255)
NSB = MID                 # 256 bwd steps (t=510..255 plus seed t=511)
CB = T + 1                # bwd score cols: 50 + inject
START, END = T - 2, T - 1
NEG = np.float32(-25000.0)  # padding; must survive float16
BIG = np.float32(60000.0)


def _host_prep2(feats, mask, transitions):
    """Build per-core device input arrays for both chains."""
    f = np.ascontiguousarray(feats, dtype=np.float32)         # (B,S,T)
    tr = np.ascontiguousarray(transitions, dtype=np.float32)  # (T,T)
    c = f.max(axis=2)                                         # (B,S)
    lengths = mask.astype(np.int64).sum(axis=1)
    lp = lengths - 1                                          # in [255,511]

    trp = np.full((TP, TP), NEG, dtype=np.float32)
    trp[:T, :T] = tr

    k = np.arange(128)
    # fwd: transP[p=(jg,b), (jl, i)] = trp[i, jg*7+jl]
    transP = np.empty((128, JL, TP), dtype=np.float32)
    for g in range(JG):
        transP[g * BL:(g + 1) * BL] = trp[:, g * JL:(g + 1) * JL].T[None]
    transP = np.ascontiguousarray(transP.reshape(128, NF))
    # bwd: transPB[p=(ig,b), (il, j)] = trp[ig*7+il, j]  (j = 0..49)
    transPB = np.empty((128, JL, T), dtype=np.float32)
    for g in range(JG):
        transPB[g * BL:(g + 1) * BL] = trp[g * JL:(g + 1) * JL, :T][None]
    transPB = np.ascontiguousarray(transPB.reshape(128, JL * T))

    REP128 = (k[:, None] % BL == k[None, :] % BL).astype(np.float16)
    G = ((np.arange(TP)[None, :] // JL) == (k[:, None] // BL)).astype(np.float32)

    # bwd recentering: cb[b,tau] = c for real steps, 0 for masked
    cb = np.where(np.arange(S)[None, :] <= lp[:, None], c, 0.0).astype(np.float32)

    per_core = []
    for ci in range(NCORES):
        sl = slice(ci * BL, (ci + 1) * BL)
        fb, cbf, cbb, lpb = f[sl], c[sl], cb[sl], lp[sl]      # per-core views

        # ---- forward arrays (t = 0..MID-1) ----
        fp = np.zeros((BL, MID, TP), dtype=np.float32)
        fp[:, :, :T] = fb[:, :MID, :] - cbf[:, :MID, None]
        fa = fp.reshape(BL, MID, JG, JL).transpose(2, 0, 1, 3).reshape(128, NF_F)
        part0 = np.full((BL, TP), NEG, dtype=np.float32)
        part0[:, :T] = fb[:, 0, :] + tr[START][None, :] - cbf[:, 0:1]
        mw0 = np.repeat(part0[None], JG, axis=0).reshape(128, TP) * G

        # ---- backward arrays: step s=1..NSB computes t = S-1-s ----
        svec = np.arange(1, NSB + 1)
        tvec = S - 1 - svec                                   # 510..255
        # featsB[p=(ig,b), (s-1)*T + j] = feats[b, t(s)+1, j] - cb[b, t(s)+1]
        fB = (fb[:, tvec + 1, :] - cbb[np.arange(BL)[:, None], tvec + 1][:, :, None])
        featsB = np.broadcast_to(fB[None], (JG, BL, NSB, T)) \
            .reshape(128, NSB * T).astype(np.float16)
        # injA[p=(ig,b), (s-1)*7+il] = trans[i(ig,il),END] if t(s)==lp else NEG
        trE = np.full(TP, NEG, dtype=np.float32)
        trE[:T] = tr[:, END]
        hit = (tvec[None, :] == lpb[:, None])                 # (BL, NSB)
        injA = np.where(
            hit[None, :, :, None],                            # (1,BL,NSB,1)
            trE.reshape(JG, 1, 1, JL),                        # (JG,1,1,JL)
            NEG).transpose(0, 1, 2, 3).reshape(JG * BL, NSB * JL)
        injA = np.ascontiguousarray(injA.astype(np.float16))
        # clampB[p, s-1] = NEG if t(s) > lp else BIG
        clampB = np.where((tvec[None, :] > lpb[:, None])[None],
                          NEG, BIG)
        clampB = np.broadcast_to(clampB, (JG, BL, NSB)) \
            .reshape(128, NSB).astype(np.float32)
        # seed beta'_{511}
        seed = np.where((lpb == S - 1)[:, None], trE[None, :T], NEG)
        seedp = np.full((BL, TP), NEG, dtype=np.float32)
        seedp[:, :T] = seed
        mw0B = np.repeat(seedp[None], JG, axis=0).reshape(128, TP) * G

        per_core.append({
            "feats_arr": np.ascontiguousarray(fa.astype(np.float16)),
            "transP": transP,
            "transPB": transPB,
            "featsB": featsB,
            "injA": injA,
            "clampB": np.ascontiguousarray(clampB),
            "I128": np.eye(128, dtype=np.float16),
            "REP128": REP128,
            "G": G,
            "mw0": np.ascontiguousarray(mw0.astype(np.float16)),
            "mw0B": np.ascontiguousarray(mw0B.astype(np.float16)),
        })
    return per_core


def build_bass():
    import concourse.bacc as bacc
    import concourse.mybir as mybir
    import concourse.tile as tile

    f32 = mybir.dt.float32
    f16 = mybir.dt.float16
    nc = bacc.Bacc("TRN2", target_bir_lowering=False, debug=False,
                   num_devices=NCORES)

    feats_d = nc.declare_dram_parameter("feats_arr", [128, NF_F], f16, isOutput=False)
    transP_d = nc.declare_dram_parameter("transP", [128, NF], f32, isOutput=False)
    transPB_d = nc.declare_dram_parameter("transPB", [128, JL * T], f32, isOutput=False)
    featsB_d = nc.declare_dram_parameter("featsB", [128, NSB * T], f16, isOutput=False)
    injA_d = nc.declare_dram_parameter("injA", [128, NSB * JL], f16, isOutput=False)
    clampB_d = nc.declare_dram_parameter("clampB", [128, NSB], f32, isOutput=False)
    i128_d = nc.declare_dram_parameter("I128", [128, 128], f16, isOutput=False)
    rep_d = nc.declare_dram_parameter("REP128", [128, 128], f16, isOutput=False)
    g_d = nc.declare_dram_parameter("G", [128, TP], f32, isOutput=False)
    mw0_d = nc.declare_dram_parameter("mw0", [128, TP], f16, isOutput=False)
    mw0B_d = nc.declare_dram_parameter("mw0B", [128, TP], f16, isOutput=False)
    hist_d = nc.declare_dram_parameter("hist", [128, NSF * JL], f16, isOutput=True)
    histB_d = nc.declare_dram_parameter("histB", [128, NSB * JL], f16, isOutput=True)

    with tile.TileContext(nc) as tc:
        with (
            tc.tile_pool(name="static", bufs=1) as sp,
            tc.tile_pool(name="state", bufs=6) as st,
            tc.tile_pool(name="psum", bufs=2, space="PSUM") as pp,
        ):
            def load(handle, shape, dt, tag):
                t_ = sp.tile(shape, dt, tag=tag)
                nc.sync.dma_start(out=t_[:, :], in_=handle[:, :])
                return t_
            # small loads first: they gate the first builds/matmuls
            transP_sb = load(transP_d, [128, NF], f32, "trP")
            transPB_sb = load(transPB_d, [128, JL * T], f32, "trPB")
            clampB_sb = load(clampB_d, [128, NSB], f32, "clampB")
            i128_sb = load(i128_d, [128, 128], f16, "i128")
            rep_sb = load(rep_d, [128, 128], f16, "rep")
            injA_sb = load(injA_d, [128, NSB * JL], f16, "injA")
            feats_sb = sp.tile([128, NF_F], f16)
            featsB_sb = sp.tile([128, NSB * T], f16)
            fchunk = NF_F // 8
            bchunk = NSB * T // 8
            for ci8 in range(8):
                nc.sync.dma_start(
                    out=featsB_sb[:, ci8 * bchunk:(ci8 + 1) * bchunk],
                    in_=featsB_d[:, ci8 * bchunk:(ci8 + 1) * bchunk])
                nc.sync.dma_start(
                    out=feats_sb[:, ci8 * fchunk:(ci8 + 1) * fchunk],
                    in_=feats_d[:, ci8 * fchunk:(ci8 + 1) * fchunk])
            g_sb = sp.tile([128, JG, JL], f32)
            nc.sync.dma_start(out=g_sb[:, :, :], in_=g_d[:, :].rearrange(
                "p (a b) -> p a b", a=JG))

            hist_sb = sp.tile([128, NSF * JL], f16)
            histB_sb = sp.tile([128, NSB * JL], f16)

            mfwF = st.tile([128, TP], f16, tag="mfwF")
            nc.sync.dma_start(out=mfwF[:, :], in_=mw0_d[:, :])
            mfwB = st.tile([128, TP], f16, tag="mfwB")
            nc.sync.dma_start(out=mfwB[:, :], in_=mw0B_d[:, :])

            transP_v = transP_sb[:, :].rearrange("p (a b) -> p a b", a=JL)
            transPB_v = transPB_sb[:, :].rearrange("p (a b) -> p a b", a=JL)

            from concourse.tile_rust import add_dep_helper

            ftF, ftB, cF, cB = {}, {}, {}, {}
            gate = [None]

            def build_F(ss):
                # fwd ft build split across the idle Act engine (4 rows via
                # Copy-activation with per-partition feats bias) and Pool
                # (3 rows), relieving the Pool bottleneck
                t_ = st.tile([128, JL, T], f16, tag="ftF%d" % (ss % 2))
                for jl in range(5):
                    nc.scalar.activation(
                        out=t_[:, jl:jl + 1, :],
                        in_=transP_v[:, jl:jl + 1, :T],
                        func=mybir.ActivationFunctionType.Identity,
                        bias=feats_sb[:, ss * JL + jl:ss * JL + jl + 1],
                        scale=1.0)
                bi = nc.gpsimd.tensor_tensor(
                    out=t_[:, 5:, :], in0=transP_v[:, 5:, :T],
                    in1=feats_sb[:, ss * JL + 5:(ss + 1) * JL].unsqueeze(2)
                    .broadcast_to([128, JL - 5, T]),
                    op=mybir.AluOpType.add)
                if gate[0] is not None:
                    add_dep_helper(bi.ins, gate[0].ins, sync=True,
                                   reason="pool after DVE mult")
                ftF[ss] = t_

            def build_B(ss):
                t_ = st.tile([128, JL, CB], f16, tag="ftB%d" % (ss % 2))
                bi = nc.gpsimd.tensor_tensor(
                    out=t_[:, :, :T], in0=transPB_v[:, :, :],
                    in1=featsB_sb[:, (ss - 1) * T:ss * T].unsqueeze(1)
                    .broadcast_to([128, JL, T]),
                    op=mybir.AluOpType.add)
                if gate[0] is not None:
                    add_dep_helper(bi.ins, gate[0].ins, sync=True,
                                   reason="pool after DVE mult")
                nc.gpsimd.tensor_copy(
                    out=t_[:, :, T:CB],
                    in_=injA_sb[:, (ss - 1) * JL:ss * JL].unsqueeze(2))
                ftB[ss] = t_

            def ftmm_F(ss):
                c_ = pp.tile([128, JL, T], f32, tag="CF%d" % (ss % 2))
                nc.tensor.matmul(c_[:, :, :], i128_sb[:, :],
                                 ftF.pop(ss)[:, :, :], start=True, stop=True)
                cF[ss] = c_

            def ftmm_B(ss):
                c_ = pp.tile([128, JL, CB], f32, tag="CB%d" % (ss % 2))
                nc.tensor.matmul(c_[:, :, :], i128_sb[:, :],
                                 ftB.pop(ss)[:, :, :], start=True, stop=True)
                cB[ss] = c_

            for ss in range(1, 4):
                build_F(ss)
                build_B(ss)
            ftmm_F(1)
            ftmm_B(1)

            for s in range(1, NSB + 1):
                run_f = s <= NSF
                # next FT matmuls first in PE order
                if s + 1 <= NSF:
                    ftmm_F(s + 1)
                if s + 1 <= NSB:
                    ftmm_B(s + 1)
                # REP matmuls
                if run_f:
                    c_f = cF.pop(s)
                    nc.tensor.matmul(
                        c_f[:, :, :], rep_sb[:, :],
                        mfwF[:, :T].unsqueeze(1).broadcast_to([128, JL, T]),
                        start=False, stop=True, skip_group_check=True)
                c_b = cB.pop(s)
                nc.tensor.matmul(
                    c_b[:, :, :T], rep_sb[:, :],
                    mfwB[:, :T].unsqueeze(1).broadcast_to([128, JL, T]),
                    start=False, stop=True, skip_group_check=True)

                # DVE: fwd reduce+mult, then bwd reduce+clamped mult
                if run_f:
                    m_f = hist_sb[:, (s - 1) * JL: s * JL]
                    nc.vector.tensor_reduce(
                        m_f, c_f[:, :, :],
                        axis=mybir.AxisListType.X, op=mybir.AluOpType.max)
                    mfwF = st.tile([128, TP], f16, tag="mfwF")
                    nc.vector.tensor_tensor(
                        out=mfwF[:, :].rearrange("p (a b) -> p a b", a=JG),
                        in0=m_f.unsqueeze(1).broadcast_to([128, JG, JL]),
                        in1=g_sb[:, :, :], op=mybir.AluOpType.mult)
                m_b = histB_sb[:, (s - 1) * JL: s * JL]
                nc.vector.tensor_reduce(
                    m_b, c_b[:, :, :],
                    axis=mybir.AxisListType.X, op=mybir.AluOpType.max)
                mfwB = st.tile([128, TP], f16, tag="mfwB")
                mult_b = nc.vector.scalar_tensor_tensor(
                    out=mfwB[:, :].rearrange("p (a b) -> p a b", a=JG),
                    in0=m_b.unsqueeze(1).broadcast_to([128, JG, JL]),
                    scalar=clampB_sb[:, s - 1:s],
                    in1=g_sb[:, :, :],
                    op0=mybir.AluOpType.min, op1=mybir.AluOpType.mult)
                gate[0] = mult_b
                if s + 3 <= NSF:
                    build_F(s + 3)
                if s + 3 <= NSB:
                    build_B(s + 3)

                if s % 16 == 0 and s < NSB:
                    lo, hi = (s - 16) * JL, s * JL
                    if s <= NSF:
                        nc.sync.dma_start(out=hist_d[:, lo:hi],
                                          in_=hist_sb[:, lo:hi])
                    nc.sync.dma_start(out=histB_d[:, lo:hi],
                                      in_=histB_sb[:, lo:hi])

            done = 240 * JL
            nc.sync.dma_start(out=hist_d[:, done:NSF * JL],
                              in_=hist_sb[:, done:NSF * JL])
            nc.sync.dma_start(out=histB_d[:, done:NSB * JL],
                              in_=histB_sb[:, done:NSB * JL])

    nc.compile()
    return nc


def _unpack(hist, nsteps):
    h = hist.astype(np.float32).reshape(JG, BL, nsteps, JL).transpose(2, 1, 0, 3)
    return h.reshape(nsteps, BL, TP)[:, :, :T]


def kernel(feats, mask, transitions):
    from concourse.bass_utils import run_bass_kernel_spmd

    feats = np.asarray(feats, dtype=np.float32)
    mask_np = np.asarray(mask).astype(bool)
    trans = np.asarray(transitions, dtype=np.float32)

    per_core = _host_prep2(feats, mask_np, trans)
    nc = build_bass()
    res = run_bass_kernel_spmd(nc, per_core, core_ids=list(range(NCORES)))

    c = feats.max(axis=2)
    lengths = mask_np.astype(np.int64).sum(axis=1)
    lp = lengths - 1
    bidx = np.arange(B)

    # assemble fwd part' (t=0..MID-1) and bwd beta' (t=MID-1..S-1)
    fwd = np.empty((MID, B, T), dtype=np.float32)
    fwd[0] = feats[:, 0, :] + trans[START][None, :] - c[:, 0:1]
    beta = np.empty((S, B, T), dtype=np.float32)
    trE = trans[:, END]
    beta[S - 1] = np.where((lp == S - 1)[:, None], trE[None, :], NEG)
    for ci in range(NCORES):
        sl = slice(ci * BL, (ci + 1) * BL)
        fwd[1:, sl] = _unpack(res.results[ci]["hist"], NSF)
        hb = _unpack(res.results[ci]["histB"], NSB)          # s=1..256
        beta[MID - 1:S - 1, sl] = hb[::-1]                    # t=255..510

    mid_tag = np.argmax(fwd[MID - 1] + beta[MID - 1], axis=1).astype(np.int32)

    decode = np.zeros((S, B), dtype=np.int32)
    decode[MID - 1] = mid_tag
    ptr = mid_tag.copy()
    trT = np.ascontiguousarray(trans.T)
    for t in range(MID - 2, -1, -1):
        sc = feats[bidx, t + 1, ptr][:, None] + trT[ptr]
        bp = np.argmax(sc + fwd[t], axis=1).astype(np.int32)
        decode[t] = bp
        ptr = bp
    tag = mid_tag.copy()
    final_tag = np.where(lp == MID - 1, mid_tag, 0).astype(np.int32)
    for t in range(MID, S):
        cur = trans[tag, :] + feats[bidx, t, :] + beta[t]
        nxt = np.argmax(cur, axis=1).astype(np.int32)
        active = t <= lp
        tag = np.where(active, nxt, tag).astype(np.int32)
        final_tag = np.where(active & (lp == t), tag, final_tag)
        decode[t] = np.where(active, tag, 0)
    decode[S - 1] = np.where(lp == S - 1, decode[S - 1], final_tag)
    return decode.T.astype(np.int32)

